# revision 27
# baseline (speedup 1.0000x reference)
"""Blinn-Phong env-map shader on 8 Trainium2 NeuronCores (Bass/Tile).

kernel(**inputs) takes the FULL inputs (shapes as in the problem's
setup_inputs) and returns the full (colors, diffuse, texels, normals)
tuple, matching the reference.

Sharding: image rows H are split across the 8 cores (32 rows each).
Every core holds the full mesh/gather tables (faces, vertex_normals) and
the full direction/env tensors, and computes its (B, 32, W, J) slice of
the diffuse intermediate on-chip.

Per-core dataflow:
  - indirect-DMA gather: faces[pix_to_face] then vertex_normals[face verts]
  - barycentric interpolation + normalize on DVE (pixel-partitioned layout)
  - mm1 on PE: raw[j, pix] = dirs . nhat  (K=3 contraction)
  - relu (== clip(x,0,1): both operands unit-norm so dot <= 1) while
    evacuating PSUM, split across ACT and DVE
  - mm2 on PE: diffuse[(b,c), pix] accumulated over j with a
    block-diagonal env lhsT
  - channel interleave via a small DRAM round-trip, multiply by texels,
    DMA out.
"""

from contextlib import ExitStack

import numpy as np

import concourse.bass as bass
import concourse.tile as tile
from concourse import bacc, mybir
from concourse.bass_utils import run_bass_kernel_spmd

F32 = mybir.dt.float32
I32 = mybir.dt.int32
P = 128

# problem constants (hardcoded per the task contract)
V, F, H, W, K, B, J = 50000, 100000, 256, 256, 1, 4, 256
NCORES = 8
HPC = H // NCORES  # rows per core
NP = HPC * W  # pixels per core
PIXTILE = 512


def build_program(
    NP=NP,
    B=B,
    J=J,
    V=V,
    F=F,
    PIXTILE=PIXTILE,
    packed=True,
    act_share=5,  # of 8 jtiles, how many relu's go to ACT (rest DVE)
):
    PPP = NP // P  # pixels per partition
    JTOT = B * J
    NJT = JTOT // P  # number of j-tiles of 128
    NPT = NP // PIXTILE  # number of pixel tiles
    assert NP % P == 0 and JTOT % P == 0 and NP % PIXTILE == 0

    nc = bacc.Bacc("TRN2", target_bir_lowering=False, debug=False)

    # --- DRAM I/O (per-core slices fed via in_maps) ---
    p2f = nc.dram_tensor("p2f", [NP], I32, kind="ExternalInput")
    bary = nc.dram_tensor("bary", [NP, 3], F32, kind="ExternalInput")
    texels = nc.dram_tensor("texels", [NP, 3], F32, kind="ExternalInput")
    faces = nc.dram_tensor("faces", [F, 3], I32, kind="ExternalInput")
    vnorm_T = nc.dram_tensor("vnorm_T", [3, V], F32, kind="ExternalInput")
    dirs = nc.dram_tensor("dirs", [JTOT, 3], F32, kind="ExternalInput")
    env = nc.dram_tensor("env", [JTOT, 3], F32, kind="ExternalInput")

    colors_o = nc.dram_tensor("colors", [B, NP, 3], F32, kind="ExternalOutput")
    diffuse_o = nc.dram_tensor("diffuse", [B, NP, 3], F32, kind="ExternalOutput")
    normals_o = nc.dram_tensor("normals", [B, NP, 3], F32, kind="ExternalOutput")
    texels_o = nc.dram_tensor("texels_out", [NP, 3], F32, kind="ExternalOutput")

    with tile.TileContext(nc) as tc, ExitStack() as ctx:
        sb = ctx.enter_context(tc.tile_pool(name="sb", bufs=1))
        dram = ctx.enter_context(tc.tile_pool(name="dram", bufs=1, space="DRAM"))
        gat_ctx = ExitStack()
        gat = gat_ctx.enter_context(tc.tile_pool(name="gat", bufs=1))
        ps1 = ctx.enter_context(tc.tile_pool(name="ps1", bufs=5, space="PSUM"))
        ps2 = ctx.enter_context(tc.tile_pool(name="ps2", bufs=2, space="PSUM"))

        # ---------- setup: loads ----------
        idx = sb.tile([P, PPP], I32)
        nc.sync.dma_start(out=idx[:], in_=p2f.ap().rearrange("(p k) -> p k", p=P))
        bary_pp = sb.tile([P, PPP * 3], F32)
        nc.sync.dma_start(
            out=bary_pp[:], in_=bary.ap().rearrange("(p k) c -> p (k c)", p=P)
        )
        texels_pp = sb.tile([P, PPP * 3], F32)
        nc.sync.dma_start(
            out=texels_pp[:], in_=texels.ap().rearrange("(p k) c -> p (k c)", p=P)
        )

        # ---------- gathers ----------
        # HW indirect DMA gathers one offset per partition per instruction
        fc = sb.tile([P, PPP * 3], I32)  # fc[p, 3k+v] = faces[p2f[pix], v]
        for k in range(PPP):
            nc.gpsimd.indirect_dma_start(
                out=fc[:, 3 * k : 3 * k + 3],
                out_offset=None,
                in_=faces.ap(),
                in_offset=bass.IndirectOffsetOnAxis(ap=idx[:, k : k + 1], axis=0),
            )
        # ---- vnorm gather via GPSIMD ap_gather (SBUF tables) ----
        # vnorm split into VH ranges of VHS rows; table row (3h+c) of each
        # 16-partition group holds plane [0, vnorm_T[c][VHS*h : VHS*(h+1)], 0]
        # (sentinel zeros at 0 and VHS+1). Gather list has 2 slots per
        # (pixel,v): idx_h = fc+1-VHS*h clamped, so out-of-range slots fetch
        # sentinel zeros and the two halves can simply be summed.
        VHS = (V + 1) // 2
        NIDX2 = 2 * PPP * 3 * 16  # per-group list length (h, v, kappa)
        vtab = gat.tile([P, VHS + 2], F32)
        nc.vector.memset(vtab[:], 0.0)
        for g in range(8):
            for h in range(2):
                for c in range(3):
                    n_rows = min(VHS, V - VHS * h)
                    nc.sync.dma_start(
                        out=vtab[16 * g + 3 * h + c : 16 * g + 3 * h + c + 1,
                                 1 : 1 + n_rows],
                        in_=vnorm_T.ap()[c, VHS * h : VHS * h + n_rows].unsqueeze(0),
                    )

        # build the wrapped index list: slot [16g+j', u2] with
        # u2 = h*(3*PPP) + v*PPP + k  <-  clamp_h(fc[16g+j', 3k+v])
        idx2_32 = gat.tile([P, NIDX2 // 16], I32)
        i2v = idx2_32[:].rearrange("p (h v k) -> p h v k", h=2, v=3)
        fcv = fc[:].rearrange("p (k v) -> p k v", v=3)
        for h in range(2):
            for v in range(3):
                # idx = fc + 1 - VHS*h, clamped to [0, VHS+1]
                nc.vector.tensor_scalar(
                    out=i2v[:, h, v, :],
                    in0=fcv[:, :, v],
                    scalar1=1 - VHS * h,
                    scalar2=0,
                    op0=mybir.AluOpType.add,
                    op1=mybir.AluOpType.max,
                )
        nc.vector.tensor_scalar_min(out=idx2_32[:], in0=idx2_32[:], scalar1=VHS + 1)
        idx2 = gat.tile([P, NIDX2 // 16], mybir.dt.int16)
        nc.vector.tensor_copy(out=idx2[:], in_=idx2_32[:])

        gout = gat.tile([P, NIDX2], F32)
        nc.gpsimd.ap_gather(
            out_ap=gout[:],
            in_ap=vtab[:],
            idxs_ap=idx2[:],
            channels=P,
            num_elems=VHS + 2,
            d=1,
            num_idxs=NIDX2,
        )
        # undo the 16-wrap: 32x32 block transpose
        gt = gat.tile([P, NIDX2], F32)
        nc.vector.transpose(out=gt[:], in_=gout[:])
        # gt[32*(g//2) + 16*(u2%2) + j', 32*(u2//2) + 16*(g%2) + row]
        #   = gout[16g+row, u2*16+j']   (row = 3h+c)
        # pixel unit (q, kap): q = partition, kap = 2*(k//2) + g%2;
        # value (h, v, c) at free: u2 = h*3*PPP + v*PPP + k ->
        #   u2//2 = h*(3*PPP//2) + v*(PPP//2) + k//2 (parities: k%2 fixed/part)
        # free = 32*(u2//2) + 16*gp + row = h*(48*PPP) + v*(16*PPP)
        #        + (k//2)*32 + gp*16 + (3h'?..c) with row=3h+c on the diagonal
        HB = 48 * PPP  # free stride of the h-half (32 * 3*PPP/2)
        VB = 16 * PPP  # free stride of v
        vn = sb.tile([P, PPP * 9], F32)  # vn[p, (v, kap, c)] = v*3*PPP + kap*3 + c
        vnv = vn[:].rearrange("p (v kap c) -> p v kap c", v=3, c=3)
        # kap*3+c ... source position for (v, kap=(2*k2+gp), c, h):
        #   h*HB + v*VB + k2*32 + gp*16 + 3h + c
        gt_ap = gt[:]
        def half_src(h):
            return bass.AP(
                tensor=gt_ap.tensor,
                offset=gt_ap.offset + h * HB + 3 * h,
                ap=[gt_ap.ap[0], [VB, 3], [16, PPP], [1, 3]],
            )
        nc.vector.tensor_add(out=vnv, in0=half_src(0), in1=half_src(1))
        gat_ctx.close()
        relu_bufs = (4 * NJT + 8) if packed else 2 * NJT
        relu_p = ctx.enter_context(tc.tile_pool(name="relu", bufs=relu_bufs))
        fin = ctx.enter_context(tc.tile_pool(name="fin", bufs=3))

        # ---------- interpolate + normalize ----------
        # vn layout is (v, kap, c); pixel unit is (partition, kap)
        vnv_ = vn[:].rearrange("p (v kap c) -> p v kap c", v=3, c=3)

        class _VNView:
            def __getitem__(self, sl):
                # emulate old [:, :, v, :] slicing -> [p, kap, c]
                _, _, v, _ = sl
                return vnv_[:, v, :, :]

        vnv = _VNView()
        baryv = bary_pp[:].rearrange("p (k v) -> p k v", v=3)
        nacc = sb.tile([P, PPP * 3], F32)
        tmp = sb.tile([P, PPP * 3], F32)
        naccv = nacc[:].rearrange("p (k c) -> p k c", c=3)
        tmpv = tmp[:].rearrange("p (k c) -> p k c", c=3)

        def bary_b(v):  # bary[p, k, v] broadcast over c -> [P, PPP, 3]
            a = baryv[:, :, v : v + 1]
            return a.to_broadcast([P, PPP, 3])

        nc.vector.tensor_tensor(
            out=naccv, in0=vnv[:, :, 0, :], in1=bary_b(0), op=mybir.AluOpType.mult
        )
        nc.vector.tensor_tensor(
            out=tmpv, in0=vnv[:, :, 1, :], in1=bary_b(1), op=mybir.AluOpType.mult
        )
        nc.vector.tensor_add(out=nacc[:], in0=nacc[:], in1=tmp[:])
        nc.vector.tensor_tensor(
            out=tmpv, in0=vnv[:, :, 2, :], in1=bary_b(2), op=mybir.AluOpType.mult
        )
        nc.vector.tensor_add(out=nacc[:], in0=nacc[:], in1=tmp[:])

        sq = sb.tile([P, PPP * 3], F32)
        nc.vector.tensor_mul(out=sq[:], in0=nacc[:], in1=nacc[:])
        ss = sb.tile([P, PPP], F32)
        nc.vector.tensor_reduce(
            out=ss[:],
            in_=sq[:].rearrange("p (k c) -> p k c", c=3),
            axis=mybir.AxisListType.X,
            op=mybir.AluOpType.add,
        )
        nrm = sb.tile([P, PPP], F32)
        nc.scalar.activation(nrm[:], ss[:], mybir.ActivationFunctionType.Sqrt)
        nc.vector.tensor_scalar_max(out=nrm[:], in0=nrm[:], scalar1=1e-6)
        rinv = sb.tile([P, PPP], F32)
        nc.vector.reciprocal(rinv[:], nrm[:])
        nhat = sb.tile([P, PPP * 3], F32)
        nc.vector.tensor_tensor(
            out=nhat[:].rearrange("p (k c) -> p k c", c=3),
            in0=naccv,
            in1=rinv[:].unsqueeze(2).to_broadcast([P, PPP, 3]),
            op=mybir.AluOpType.mult,
        )

        # normals output: nhat_pp is already (pix-part, interleaved ch) layout
        no_v = normals_o.ap().rearrange("b (p k) c -> b p (k c)", p=P)
        for b in range(B):
            nc.sync.dma_start(out=no_v[b], in_=nhat[:])

        # texels passthrough (DRAM->DRAM)
        nc.sync.dma_start(out=texels_o.ap(), in_=texels.ap())

        # ---------- nhat -> channel-planar [3, NP] via DRAM round-trip ----------
        nplan = sb.tile([P, PPP * 3], F32)  # [p, (c k)] channel-major
        nhatv = nhat[:].rearrange("p (k c) -> p k c", c=3)
        nplanv = nplan[:].rearrange("p (c k) -> p c k", c=3)
        for c in range(3):
            nc.vector.tensor_copy(out=nplanv[:, c, :], in_=nhatv[:, :, c])
        n_sc = dram.tile([P, 3 * PPP], F32)
        nc.sync.dma_start(out=n_sc[:], in_=nplan[:])
        # read back as [3, NP]: addr = p*(3*PPP) + c*PPP + k ; pix = p*PPP+k
        # with packing, replicate at partition bases 0/32/64/96 (row groups)
        NREP = 4 if packed else 1
        n_cp = sb.tile([P if packed else 3, NP], F32)
        n_sc_ap = n_sc[:]
        for i in range(NREP):
            nc.sync.dma_start(
                out=n_cp[32 * i : 32 * i + 3, :].rearrange(
                    "c (p k) -> c p k", p=P
                ),
                in_=bass.AP(
                    tensor=n_sc.tensor,
                    offset=n_sc_ap.offset,
                    ap=[[PPP, 3], [3 * PPP, P], [1, PPP]],
                ),
            )

        # ---------- dirs -> [3, JTOT] via DRAM round-trip ----------
        RPP = JTOT // P  # dir rows per partition
        dsb = sb.tile([P, RPP * 3], F32)
        nc.sync.dma_start(
            out=dsb[:], in_=dirs.ap().rearrange("(p r) c -> p (r c)", p=P)
        )
        dplan = sb.tile([P, 3 * RPP], F32)
        dsbv = dsb[:].rearrange("p (r c) -> p r c", c=3)
        dplanv = dplan[:].rearrange("p (c r) -> p c r", c=3)
        for c in range(3):
            nc.vector.tensor_copy(out=dplanv[:, c, :], in_=dsbv[:, :, c])
        d_sc = dram.tile([P, 3 * RPP], F32)
        nc.sync.dma_start(out=d_sc[:], in_=dplan[:])
        dirsT = sb.tile([P if packed else 3, JTOT], F32)
        d_sc_ap = d_sc[:]
        for i in range(NREP):
            nc.sync.dma_start(
                out=dirsT[32 * i : 32 * i + 3, :].rearrange(
                    "c (p r) -> c p r", p=P
                ),
                in_=bass.AP(
                    tensor=d_sc.tensor,
                    offset=d_sc_ap.offset,
                    ap=[[RPP, 3], [3 * RPP, P], [1, RPP]],
                ),
            )

        # ---------- env block-diagonal lhsT tiles [128, 3B] ----------
        env_bd = []
        MBD = 32 if packed else 3 * B  # pad M so packed mm2 fills partitions
        for q in range(NJT):
            t = sb.tile([P, MBD], F32, tag=f"envbd{q}")
            nc.vector.memset(t[:], 0.0)
            env_bd.append(t)
        # chunk q covers flat dirs [128q, 128q+128); row i belongs to batch
        # (128q+i)//J and goes to cols 3b..3b+3
        for q in range(NJT):
            lo = q * P
            hi = lo + P
            stage = sb.tile([P, 3], F32, name=f"envstage{q}", tag=f"envstage{q}")
            nc.sync.dma_start(out=stage[:], in_=env.ap()[lo:hi, :])
            b0, b1 = lo // J, (hi - 1) // J
            for b in range(b0, b1 + 1):
                rlo = max(lo, b * J) - lo
                rhi = min(hi, (b + 1) * J) - lo
                nc.vector.tensor_copy(
                    out=env_bd[q][rlo:rhi, 3 * b : 3 * b + 3],
                    in_=stage[rlo:rhi, :],
                )

        # ---------- diffuse DRAM scratch (planar [3B, NP]) ----------
        diff_sc = dram.tile([3 * B, NP], F32)

        # ---------- main loop ----------
        def relu_engine(seq):
            if seq % NJT < act_share:
                return "act"
            return "vec"

        def do_relu(rt, pm, seq):
            if relu_engine(seq) == "act":
                nc.scalar.activation(rt[:], pm[:], mybir.ActivationFunctionType.Relu)
            else:
                nc.vector.tensor_scalar_max(out=rt[:], in0=pm[:], scalar1=0.0)

        if not packed:
            for pt in range(NPT):
                psl = slice(pt * PIXTILE, (pt + 1) * PIXTILE)
                relu_t = []
                for jt in range(NJT):
                    pm = ps1.tile([P, PIXTILE], F32, tag="mm1")
                    nc.tensor.matmul(
                        out=pm[:],
                        lhsT=dirsT[:3, jt * P : (jt + 1) * P],
                        rhs=n_cp[:3, psl],
                        start=True,
                        stop=True,
                    )
                    rt = relu_p.tile([P, PIXTILE], F32, tag="relu")
                    do_relu(rt, pm, pt * NJT + jt)
                    relu_t.append(rt)
                pm2 = ps2.tile([3 * B, PIXTILE], F32, tag="mm2")
                for q in range(NJT):
                    nc.tensor.matmul(
                        out=pm2[:],
                        lhsT=env_bd[q][:],
                        rhs=relu_t[q][:],
                        start=(q == 0),
                        stop=(q == NJT - 1),
                    )
                dsb2 = fin.tile([3 * B, PIXTILE], F32, tag="devac")
                nc.scalar.copy(out=dsb2[:], in_=pm2[:])
                nc.sync.dma_start(out=diff_sc[:, psl], in_=dsb2[:])
        else:
            # pixtile groups of GRP; mm1 in 16-tile waves (4 row-groups x 4
            # col-groups), mm2 col-packed across the group's pixtiles.
            GRP = 4
            NW = NJT // 4  # mm1 waves per pixtile (each wave = 4 jtiles)
            assert NJT % 4 == 0 and NPT % GRP == 0
            for ptg in range(NPT // GRP):
                relu_t = {}  # (pt_in_grp, jt) -> tile
                for g in range(GRP):
                    pt = ptg * GRP + g
                    psl = slice(pt * PIXTILE, (pt + 1) * PIXTILE)
                    for w in range(NW):
                        pms = [
                            ps1.tile([P, PIXTILE], F32, tag="mm1", name=f"pm{i}")
                            for i in range(4)
                        ]
                        # emit col-group-inner/row-group-outer so LDWEIGHTS
                        # of the next row-group overlaps in-flight matmuls
                        for jj in range(4):
                            for i in range(4):
                                jt = 4 * w + i
                                off = P * jt + 32 * jj
                                nc.tensor.matmul(
                                    out=pms[i][32 * jj : 32 * jj + 32, :],
                                    lhsT=dirsT[
                                        32 * i : 32 * i + 3, off : off + 32
                                    ],
                                    rhs=n_cp[32 * i : 32 * i + 3, psl],
                                    start=True,
                                    stop=True,
                                    tile_position=(32 * i, 32 * jj),
                                )
                        for i in range(4):
                            jt = 4 * w + i
                            rt = relu_p.tile([P, PIXTILE], F32, tag="relu")
                            do_relu(rt, pms[i], pt * NJT + jt)
                            relu_t[(g, jt)] = rt
                pm2 = ps2.tile([P, PIXTILE], F32, tag="mm2")
                for q in range(NJT):
                    for g in range(GRP):
                        nc.tensor.matmul(
                            out=pm2[32 * g : 32 * g + 32, :],
                            lhsT=env_bd[q][:],
                            rhs=relu_t[(g, q)][:],
                            start=(q == 0),
                            stop=(q == NJT - 1),
                            tile_position=(0, 32 * g),
                            skip_group_check=True,
                        )
                dsb2 = fin.tile([P, PIXTILE], F32, tag="devac")
                nc.scalar.copy(out=dsb2[:], in_=pm2[:])
                # strip g rows [32g, 32g+12) -> diff_sc[:, pixtile ptg*GRP+g]
                for g in range(GRP):
                    pt = ptg * GRP + g
                    nc.sync.dma_start(
                        out=diff_sc[:, pt * PIXTILE : (pt + 1) * PIXTILE],
                        in_=dsb2[32 * g : 32 * g + 3 * B, :],
                    )

        # ---------- final: interleave + colors ----------
        # diff_sc[3b+c, pix], pix = p*PPP + k
        dscv = diff_sc[:]
        for b in range(B):
            din = fin.tile([P, 3 * PPP], F32, tag="din")  # [p, (c k)]
            src = bass.AP(
                tensor=diff_sc.tensor,
                offset=dscv.offset + 3 * b * NP,
                ap=[[PPP, P], [NP, 3], [1, PPP]],
            )
            nc.sync.dma_start(out=din[:], in_=src)
            dint = fin.tile([P, PPP * 3], F32, tag="dint")
            dinv = din[:].rearrange("p (c k) -> p c k", c=3)
            dintv = dint[:].rearrange("p (k c) -> p k c", c=3)
            for c in range(3):
                nc.vector.tensor_copy(out=dintv[:, :, c], in_=dinv[:, c, :])
            cint = fin.tile([P, PPP * 3], F32, tag="cint")
            nc.vector.tensor_mul(out=cint[:], in0=dint[:], in1=texels_pp[:])
            do_v = diffuse_o.ap().rearrange("b (p k) c -> b p (k c)", p=P)
            co_v = colors_o.ap().rearrange("b (p k) c -> b p (k c)", p=P)
            nc.sync.dma_start(out=do_v[b], in_=dint[:])
            nc.sync.dma_start(out=co_v[b], in_=cint[:])

    nc.compile()
    return nc


_PROGRAM_CACHE = {}


def _get_program():
    if "nc" not in _PROGRAM_CACHE:
        _PROGRAM_CACHE["nc"] = build_program()
    return _PROGRAM_CACHE["nc"]


def _sigma_perm():
    """Pixel permutation induced by the vn gather's 16-wrap + 32x32
    transpose: device pixel-unit (partition q, slot kap) <-> core-local
    image pixel sigma[q*PPP_ + kap]."""
    PPP_ = NP // P
    q = np.arange(P)[:, None]
    kap = np.arange(PPP_)[None, :]
    a = q // 32
    e = (q // 16) % 2
    jp = q % 16
    k2 = kap // 2
    w = kap % 2
    g = 2 * a + w
    k = 2 * k2 + e
    return ((16 * g + jp) * PPP_ + k).reshape(-1)


_SIGMA = _sigma_perm()


def make_in_maps(
    pix_to_face, bary_coords, faces, verts, vertex_normals, directions,
    env_colors, texels,
):
    del verts  # unused by the computation
    in_maps = []
    faces_c = np.ascontiguousarray(faces, dtype=np.int32)
    vnormT_c = np.ascontiguousarray(
        np.asarray(vertex_normals, dtype=np.float32).T
    )
    dirs_c = np.ascontiguousarray(directions, dtype=np.float32).reshape(B * J, 3)
    env_c = np.ascontiguousarray(env_colors, dtype=np.float32).reshape(B * J, 3)
    for r in range(NCORES):
        rs = slice(r * HPC, (r + 1) * HPC)
        bary_r = np.asarray(bary_coords[0, rs, :, 0, :], dtype=np.float32).reshape(
            NP, 3
        )
        tex_r = np.asarray(texels[0, rs, :, :], dtype=np.float32).reshape(NP, 3)
        in_maps.append(
            {
                "p2f": np.ascontiguousarray(
                    pix_to_face[0, rs, :, 0], dtype=np.int32
                ).reshape(NP),
                "bary": np.ascontiguousarray(bary_r[_SIGMA]),
                "texels": np.ascontiguousarray(tex_r[_SIGMA]),
                "faces": faces_c,
                "vnorm_T": vnormT_c,
                "dirs": dirs_c,
                "env": env_c,
            }
        )
    return in_maps


def assemble_outputs(results):
    colors = np.empty((B, H, W, 3), dtype=np.float32)
    diffuse = np.empty((B, H, W, 3), dtype=np.float32)
    normals = np.empty((B, H, W, 3), dtype=np.float32)
    texels = np.empty((1, H, W, 3), dtype=np.float32)
    inv = np.empty_like(_SIGMA)
    inv[_SIGMA] = np.arange(NP)
    for r in range(NCORES):
        rs = slice(r * HPC, (r + 1) * HPC)
        colors[:, rs] = results[r]["colors"][:, inv].reshape(B, HPC, W, 3)
        diffuse[:, rs] = results[r]["diffuse"][:, inv].reshape(B, HPC, W, 3)
        normals[:, rs] = results[r]["normals"][:, inv].reshape(B, HPC, W, 3)
        texels[0, rs] = results[r]["texels_out"][inv].reshape(HPC, W, 3)
    return colors, diffuse, texels, normals


def run_on_hw(in_maps, trace=False):
    nc = _get_program()
    return run_bass_kernel_spmd(nc, in_maps, list(range(NCORES)), trace=trace)


def kernel(**inputs):
    in_maps = make_in_maps(**inputs)
    res = run_on_hw(in_maps)
    return assemble_outputs(res.results)


# revision 31
# speedup vs baseline: 1.0804x; 1.0804x over previous
"""Blinn-Phong env-map shader on 8 Trainium2 NeuronCores (Bass/Tile).

kernel(**inputs) takes the FULL inputs (shapes as in the problem's
setup_inputs) and returns the full (colors, diffuse, texels, normals)
tuple, matching the reference.

Sharding: image rows H are split across the 8 cores (32 rows each).
Every core holds the full mesh/gather tables (faces, vertex_normals) and
the full direction/env tensors, and computes its (B, 32, W, J) slice of
the diffuse intermediate on-chip.

Per-core dataflow:
  - indirect-DMA gather: faces[pix_to_face] then vertex_normals[face verts]
  - barycentric interpolation + normalize on DVE (pixel-partitioned layout)
  - mm1 on PE: raw[j, pix] = dirs . nhat  (K=3 contraction)
  - relu (== clip(x,0,1): both operands unit-norm so dot <= 1) while
    evacuating PSUM, split across ACT and DVE
  - mm2 on PE: diffuse[(b,c), pix] accumulated over j with a
    block-diagonal env lhsT
  - channel interleave via a small DRAM round-trip, multiply by texels,
    DMA out.
"""

from contextlib import ExitStack

import numpy as np

import concourse.bass as bass
import concourse.tile as tile
from concourse import bacc, mybir
from concourse.bass_utils import run_bass_kernel_spmd

F32 = mybir.dt.float32
I32 = mybir.dt.int32
P = 128

# problem constants (hardcoded per the task contract)
V, F, H, W, K, B, J = 50000, 100000, 256, 256, 1, 4, 256
NCORES = 8
HPC = H // NCORES  # rows per core
NP = HPC * W  # pixels per core
PIXTILE = 512


def build_program(
    NP=NP,
    B=B,
    J=J,
    V=V,
    F=F,
    PIXTILE=PIXTILE,
    packed=True,
    act_share=5,  # of 8 jtiles, how many relu's go to ACT (rest DVE)
):
    PPP = NP // P  # pixels per partition
    JTOT = B * J
    NJT = JTOT // P  # number of j-tiles of 128
    NPT = NP // PIXTILE  # number of pixel tiles
    assert NP % P == 0 and JTOT % P == 0 and NP % PIXTILE == 0

    nc = bacc.Bacc("TRN2", target_bir_lowering=False, debug=False,
                   dynamic_dma_scratch_size=28672)

    # --- DRAM I/O (per-core slices fed via in_maps) ---
    p2f = nc.dram_tensor("p2f", [NP], I32, kind="ExternalInput")
    bary = nc.dram_tensor("bary", [NP, 3], F32, kind="ExternalInput")
    texels = nc.dram_tensor("texels", [NP, 3], F32, kind="ExternalInput")
    faces = nc.dram_tensor("faces", [F, 3], I32, kind="ExternalInput")
    vnorm_T = nc.dram_tensor("vnorm_T", [3, V], F32, kind="ExternalInput")
    dirs = nc.dram_tensor("dirs", [JTOT, 3], F32, kind="ExternalInput")
    env = nc.dram_tensor("env", [JTOT, 3], F32, kind="ExternalInput")

    colors_o = nc.dram_tensor("colors", [B, NP, 3], F32, kind="ExternalOutput")
    diffuse_o = nc.dram_tensor("diffuse", [B, NP, 3], F32, kind="ExternalOutput")
    normals_o = nc.dram_tensor("normals", [B, NP, 3], F32, kind="ExternalOutput")
    texels_o = nc.dram_tensor("texels_out", [NP, 3], F32, kind="ExternalOutput")

    with tile.TileContext(nc) as tc, ExitStack() as ctx:
        sb = ctx.enter_context(tc.tile_pool(name="sb", bufs=1))
        dram = ctx.enter_context(tc.tile_pool(name="dram", bufs=1, space="DRAM"))
        gat_ctx = ExitStack()
        gat = gat_ctx.enter_context(tc.tile_pool(name="gat", bufs=1))
        ps1 = ctx.enter_context(tc.tile_pool(name="ps1", bufs=5, space="PSUM"))
        ps2 = ctx.enter_context(tc.tile_pool(name="ps2", bufs=2, space="PSUM"))

        # ---------- setup: loads ----------
        idx = sb.tile([P, PPP], I32)
        nc.sync.dma_start(out=idx[:], in_=p2f.ap().rearrange("(p k) -> p k", p=P))
        bary_pp = sb.tile([P, PPP * 3], F32)
        nc.sync.dma_start(
            out=bary_pp[:], in_=bary.ap().rearrange("(p k) c -> p (k c)", p=P)
        )
        texels_pp = sb.tile([P, PPP * 3], F32)
        nc.sync.dma_start(
            out=texels_pp[:], in_=texels.ap().rearrange("(p k) c -> p (k c)", p=P)
        )

        # ---------- gathers ----------
        # HW indirect DMA gathers one offset per partition per instruction
        fc = sb.tile([P, PPP * 3], I32)  # fc[p, 3k+v] = faces[p2f[pix], v]
        for k in range(PPP):
            nc.gpsimd.indirect_dma_start(
                out=fc[:, 3 * k : 3 * k + 3],
                out_offset=None,
                in_=faces.ap(),
                in_offset=bass.IndirectOffsetOnAxis(ap=idx[:, k : k + 1], axis=0),
            )
        # ---- vnorm gather via GPSIMD ap_gather (SBUF tables) ----
        # vnorm split into VH ranges of VHS rows; table row (3h+c) of each
        # 16-partition group holds plane [0, vnorm_T[c][VHS*h : VHS*(h+1)], 0]
        # (sentinel zeros at 0 and VHS+1). Gather list has 2 slots per
        # (pixel,v): idx_h = fc+1-VHS*h clamped, so out-of-range slots fetch
        # sentinel zeros and the two halves can simply be summed.
        VHS = (V + 1) // 2
        NIDX2 = 2 * PPP * 3 * 16  # per-group list length (h, v, kappa)
        vtab = gat.tile([P, VHS + 2], F32)
        nc.vector.memset(vtab[:], 0.0)
        for g in range(8):
            for h in range(2):
                for c in range(3):
                    n_rows = min(VHS, V - VHS * h)
                    nc.sync.dma_start(
                        out=vtab[16 * g + 3 * h + c : 16 * g + 3 * h + c + 1,
                                 1 : 1 + n_rows],
                        in_=vnorm_T.ap()[c, VHS * h : VHS * h + n_rows].unsqueeze(0),
                    )

        # build the wrapped index list: slot [16g+j', u2] with
        # u2 = h*(3*PPP) + v*PPP + k  <-  clamp_h(fc[16g+j', 3k+v])
        idx2_32 = gat.tile([P, NIDX2 // 16], I32)
        i2v = idx2_32[:].rearrange("p (h v k) -> p h v k", h=2, v=3)
        fcv = fc[:].rearrange("p (k v) -> p k v", v=3)
        for h in range(2):
            for v in range(3):
                # idx = fc + 1 - VHS*h, clamped to [0, VHS+1]
                nc.vector.tensor_scalar(
                    out=i2v[:, h, v, :],
                    in0=fcv[:, :, v],
                    scalar1=1 - VHS * h,
                    scalar2=0,
                    op0=mybir.AluOpType.add,
                    op1=mybir.AluOpType.max,
                )
        nc.vector.tensor_scalar_min(out=idx2_32[:], in0=idx2_32[:], scalar1=VHS + 1)
        idx2 = gat.tile([P, NIDX2 // 16], mybir.dt.int16)
        nc.vector.tensor_copy(out=idx2[:], in_=idx2_32[:])

        gout = gat.tile([P, NIDX2], F32)
        nc.gpsimd.ap_gather(
            out_ap=gout[:],
            in_ap=vtab[:],
            idxs_ap=idx2[:],
            channels=P,
            num_elems=VHS + 2,
            d=1,
            num_idxs=NIDX2,
        )
        # undo the 16-wrap: 32x32 block transpose
        gt = gat.tile([P, NIDX2], F32)
        nc.vector.transpose(out=gt[:], in_=gout[:])
        # gt[32*(g//2) + 16*(u2%2) + j', 32*(u2//2) + 16*(g%2) + row]
        #   = gout[16g+row, u2*16+j']   (row = 3h+c)
        # pixel unit (q, kap): q = partition, kap = 2*(k//2) + g%2;
        # value (h, v, c) at free: u2 = h*3*PPP + v*PPP + k ->
        #   u2//2 = h*(3*PPP//2) + v*(PPP//2) + k//2 (parities: k%2 fixed/part)
        # free = 32*(u2//2) + 16*gp + row = h*(48*PPP) + v*(16*PPP)
        #        + (k//2)*32 + gp*16 + (3h'?..c) with row=3h+c on the diagonal
        HB = 48 * PPP  # free stride of the h-half (32 * 3*PPP/2)
        VB = 16 * PPP  # free stride of v
        vn = sb.tile([P, PPP * 9], F32)  # vn[p, (v, kap, c)] = v*3*PPP + kap*3 + c
        vnv = vn[:].rearrange("p (v kap c) -> p v kap c", v=3, c=3)
        # kap*3+c ... source position for (v, kap=(2*k2+gp), c, h):
        #   h*HB + v*VB + k2*32 + gp*16 + 3h + c
        gt_ap = gt[:]
        def half_src(h):
            return bass.AP(
                tensor=gt_ap.tensor,
                offset=gt_ap.offset + h * HB + 3 * h,
                ap=[gt_ap.ap[0], [VB, 3], [16, PPP], [1, 3]],
            )
        nc.vector.tensor_add(out=vnv, in0=half_src(0), in1=half_src(1))
        gat_ctx.close()
        relu_bufs = (4 * NJT + 8) if packed else 2 * NJT
        relu_p = ctx.enter_context(tc.tile_pool(name="relu", bufs=relu_bufs))
        fin = ctx.enter_context(tc.tile_pool(name="fin", bufs=3))

        # ---------- interpolate + normalize ----------
        # vn layout is (v, kap, c); pixel unit is (partition, kap)
        vnv_ = vn[:].rearrange("p (v kap c) -> p v kap c", v=3, c=3)

        class _VNView:
            def __getitem__(self, sl):
                # emulate old [:, :, v, :] slicing -> [p, kap, c]
                _, _, v, _ = sl
                return vnv_[:, v, :, :]

        vnv = _VNView()
        baryv = bary_pp[:].rearrange("p (k v) -> p k v", v=3)
        nacc = sb.tile([P, PPP * 3], F32)
        tmp = sb.tile([P, PPP * 3], F32)
        naccv = nacc[:].rearrange("p (k c) -> p k c", c=3)
        tmpv = tmp[:].rearrange("p (k c) -> p k c", c=3)

        def bary_b(v):  # bary[p, k, v] broadcast over c -> [P, PPP, 3]
            a = baryv[:, :, v : v + 1]
            return a.to_broadcast([P, PPP, 3])

        nc.vector.tensor_tensor(
            out=naccv, in0=vnv[:, :, 0, :], in1=bary_b(0), op=mybir.AluOpType.mult
        )
        nc.vector.tensor_tensor(
            out=tmpv, in0=vnv[:, :, 1, :], in1=bary_b(1), op=mybir.AluOpType.mult
        )
        nc.vector.tensor_add(out=nacc[:], in0=nacc[:], in1=tmp[:])
        nc.vector.tensor_tensor(
            out=tmpv, in0=vnv[:, :, 2, :], in1=bary_b(2), op=mybir.AluOpType.mult
        )
        nc.vector.tensor_add(out=nacc[:], in0=nacc[:], in1=tmp[:])

        sq = sb.tile([P, PPP * 3], F32)
        nc.vector.tensor_mul(out=sq[:], in0=nacc[:], in1=nacc[:])
        ss = sb.tile([P, PPP], F32)
        nc.vector.tensor_reduce(
            out=ss[:],
            in_=sq[:].rearrange("p (k c) -> p k c", c=3),
            axis=mybir.AxisListType.X,
            op=mybir.AluOpType.add,
        )
        nrm = sb.tile([P, PPP], F32)
        nc.scalar.activation(nrm[:], ss[:], mybir.ActivationFunctionType.Sqrt)
        nc.vector.tensor_scalar_max(out=nrm[:], in0=nrm[:], scalar1=1e-6)
        rinv = sb.tile([P, PPP], F32)
        nc.vector.reciprocal(rinv[:], nrm[:])
        nhat = sb.tile([P, PPP * 3], F32)
        nc.vector.tensor_tensor(
            out=nhat[:].rearrange("p (k c) -> p k c", c=3),
            in0=naccv,
            in1=rinv[:].unsqueeze(2).to_broadcast([P, PPP, 3]),
            op=mybir.AluOpType.mult,
        )

        # normals output: nhat_pp is already (pix-part, interleaved ch) layout
        no_v = normals_o.ap().rearrange("b (p k) c -> b p (k c)", p=P)
        for b in range(B):
            nc.sync.dma_start(out=no_v[b], in_=nhat[:])

        # texels passthrough (DRAM->DRAM)
        nc.sync.dma_start(out=texels_o.ap(), in_=texels.ap())

        # ---------- nhat -> channel-planar [3, NP] via DRAM round-trip ----------
        nplan = sb.tile([P, PPP * 3], F32)  # [p, (c k)] channel-major
        nhatv = nhat[:].rearrange("p (k c) -> p k c", c=3)
        nplanv = nplan[:].rearrange("p (c k) -> p c k", c=3)
        for c in range(3):
            nc.vector.tensor_copy(out=nplanv[:, c, :], in_=nhatv[:, :, c])
        n_sc = dram.tile([P, 3 * PPP], F32)
        nc.sync.dma_start(out=n_sc[:], in_=nplan[:])
        # read back as [3, NP]: addr = p*(3*PPP) + c*PPP + k ; pix = p*PPP+k
        # with packing, replicate at partition bases 0/32/64/96 (row groups)
        NREP = 4 if packed else 1
        n_cp = sb.tile([P if packed else 3, NP], F32)
        n_sc_ap = n_sc[:]
        for i in range(NREP):
            nc.sync.dma_start(
                out=n_cp[32 * i : 32 * i + 3, :].rearrange(
                    "c (p k) -> c p k", p=P
                ),
                in_=bass.AP(
                    tensor=n_sc.tensor,
                    offset=n_sc_ap.offset,
                    ap=[[PPP, 3], [3 * PPP, P], [1, PPP]],
                ),
            )

        # ---------- dirs -> [3, JTOT] via DRAM round-trip ----------
        RPP = JTOT // P  # dir rows per partition
        dsb = sb.tile([P, RPP * 3], F32)
        nc.sync.dma_start(
            out=dsb[:], in_=dirs.ap().rearrange("(p r) c -> p (r c)", p=P)
        )
        dplan = sb.tile([P, 3 * RPP], F32)
        dsbv = dsb[:].rearrange("p (r c) -> p r c", c=3)
        dplanv = dplan[:].rearrange("p (c r) -> p c r", c=3)
        for c in range(3):
            nc.vector.tensor_copy(out=dplanv[:, c, :], in_=dsbv[:, :, c])
        d_sc = dram.tile([P, 3 * RPP], F32)
        nc.sync.dma_start(out=d_sc[:], in_=dplan[:])
        dirsT = sb.tile([P if packed else 3, JTOT], F32)
        d_sc_ap = d_sc[:]
        for i in range(NREP):
            nc.sync.dma_start(
                out=dirsT[32 * i : 32 * i + 3, :].rearrange(
                    "c (p r) -> c p r", p=P
                ),
                in_=bass.AP(
                    tensor=d_sc.tensor,
                    offset=d_sc_ap.offset,
                    ap=[[RPP, 3], [3 * RPP, P], [1, RPP]],
                ),
            )

        # ---------- env block-diagonal lhsT tiles [128, 3B] ----------
        env_bd = []
        MBD = 32 if packed else 3 * B  # pad M so packed mm2 fills partitions
        for q in range(NJT):
            t = sb.tile([P, MBD], F32, tag=f"envbd{q}")
            nc.vector.memset(t[:], 0.0)
            env_bd.append(t)
        # chunk q covers flat dirs [128q, 128q+128); row i belongs to batch
        # (128q+i)//J and goes to cols 3b..3b+3
        for q in range(NJT):
            lo = q * P
            hi = lo + P
            stage = sb.tile([P, 3], F32, name=f"envstage{q}", tag=f"envstage{q}")
            nc.sync.dma_start(out=stage[:], in_=env.ap()[lo:hi, :])
            b0, b1 = lo // J, (hi - 1) // J
            for b in range(b0, b1 + 1):
                rlo = max(lo, b * J) - lo
                rhi = min(hi, (b + 1) * J) - lo
                nc.vector.tensor_copy(
                    out=env_bd[q][rlo:rhi, 3 * b : 3 * b + 3],
                    in_=stage[rlo:rhi, :],
                )

        # ---------- diffuse DRAM scratch (planar [3B, NP]) ----------
        diff_sc = dram.tile([3 * B, NP], F32)

        # ---------- main loop ----------
        def relu_engine(seq):
            if seq % NJT < act_share:
                return "act"
            return "vec"

        def do_relu(rt, pm, seq):
            if relu_engine(seq) == "act":
                nc.scalar.activation(rt[:], pm[:], mybir.ActivationFunctionType.Relu)
            else:
                nc.vector.tensor_scalar_max(out=rt[:], in0=pm[:], scalar1=0.0)

        if not packed:
            for pt in range(NPT):
                psl = slice(pt * PIXTILE, (pt + 1) * PIXTILE)
                relu_t = []
                for jt in range(NJT):
                    pm = ps1.tile([P, PIXTILE], F32, tag="mm1")
                    nc.tensor.matmul(
                        out=pm[:],
                        lhsT=dirsT[:3, jt * P : (jt + 1) * P],
                        rhs=n_cp[:3, psl],
                        start=True,
                        stop=True,
                    )
                    rt = relu_p.tile([P, PIXTILE], F32, tag="relu")
                    do_relu(rt, pm, pt * NJT + jt)
                    relu_t.append(rt)
                pm2 = ps2.tile([3 * B, PIXTILE], F32, tag="mm2")
                for q in range(NJT):
                    nc.tensor.matmul(
                        out=pm2[:],
                        lhsT=env_bd[q][:],
                        rhs=relu_t[q][:],
                        start=(q == 0),
                        stop=(q == NJT - 1),
                    )
                dsb2 = fin.tile([3 * B, PIXTILE], F32, tag="devac")
                nc.scalar.copy(out=dsb2[:], in_=pm2[:])
                nc.sync.dma_start(out=diff_sc[:, psl], in_=dsb2[:])
        else:
            # pixtile groups of GRP; mm1 in 16-tile waves (4 row-groups x 4
            # col-groups), mm2 col-packed across the group's pixtiles.
            GRP = 4
            NW = NJT // 4  # mm1 waves per pixtile (each wave = 4 jtiles)
            assert NJT % 4 == 0 and NPT % GRP == 0
            for ptg in range(NPT // GRP):
                relu_t = {}  # (pt_in_grp, jt) -> tile
                for g in range(GRP):
                    pt = ptg * GRP + g
                    psl = slice(pt * PIXTILE, (pt + 1) * PIXTILE)
                    for w in range(NW):
                        pms = [
                            ps1.tile([P, PIXTILE], F32, tag="mm1", name=f"pm{i}")
                            for i in range(4)
                        ]
                        # emit col-group-inner/row-group-outer so LDWEIGHTS
                        # of the next row-group overlaps in-flight matmuls
                        for jj in range(4):
                            for i in range(4):
                                jt = 4 * w + i
                                off = P * jt + 32 * jj
                                nc.tensor.matmul(
                                    out=pms[i][32 * jj : 32 * jj + 32, :],
                                    lhsT=dirsT[
                                        32 * i : 32 * i + 3, off : off + 32
                                    ],
                                    rhs=n_cp[32 * i : 32 * i + 3, psl],
                                    start=True,
                                    stop=True,
                                    tile_position=(32 * i, 32 * jj),
                                )
                        for i in range(4):
                            jt = 4 * w + i
                            rt = relu_p.tile([P, PIXTILE], F32, tag="relu")
                            do_relu(rt, pms[i], pt * NJT + jt)
                            relu_t[(g, jt)] = rt
                pm2 = ps2.tile([P, PIXTILE], F32, tag="mm2")
                for q in range(NJT):
                    for g in range(GRP):
                        nc.tensor.matmul(
                            out=pm2[32 * g : 32 * g + 32, :],
                            lhsT=env_bd[q][:],
                            rhs=relu_t[(g, q)][:],
                            start=(q == 0),
                            stop=(q == NJT - 1),
                            tile_position=(0, 32 * g),
                            skip_group_check=True,
                        )
                dsb2 = fin.tile([P, PIXTILE], F32, tag="devac")
                nc.scalar.copy(out=dsb2[:], in_=pm2[:])
                # strip g rows [32g, 32g+12) -> diff_sc[:, pixtile ptg*GRP+g]
                for g in range(GRP):
                    pt = ptg * GRP + g
                    nc.sync.dma_start(
                        out=diff_sc[:, pt * PIXTILE : (pt + 1) * PIXTILE],
                        in_=dsb2[32 * g : 32 * g + 3 * B, :],
                    )

        # ---------- final: interleave + colors ----------
        # diff_sc[3b+c, pix], pix = p*PPP + k
        dscv = diff_sc[:]
        for b in range(B):
            din = fin.tile([P, 3 * PPP], F32, tag="din")  # [p, (c k)]
            src = bass.AP(
                tensor=diff_sc.tensor,
                offset=dscv.offset + 3 * b * NP,
                ap=[[PPP, P], [NP, 3], [1, PPP]],
            )
            nc.sync.dma_start(out=din[:], in_=src)
            dint = fin.tile([P, PPP * 3], F32, tag="dint")
            dinv = din[:].rearrange("p (c k) -> p c k", c=3)
            dintv = dint[:].rearrange("p (k c) -> p k c", c=3)
            for c in range(3):
                nc.vector.tensor_copy(out=dintv[:, :, c], in_=dinv[:, c, :])
            cint = fin.tile([P, PPP * 3], F32, tag="cint")
            nc.vector.tensor_mul(out=cint[:], in0=dint[:], in1=texels_pp[:])
            do_v = diffuse_o.ap().rearrange("b (p k) c -> b p (k c)", p=P)
            co_v = colors_o.ap().rearrange("b (p k) c -> b p (k c)", p=P)
            nc.sync.dma_start(out=do_v[b], in_=dint[:])
            nc.sync.dma_start(out=co_v[b], in_=cint[:])

    nc.compile()
    return nc


_PROGRAM_CACHE = {}


def _get_program():
    if "nc" not in _PROGRAM_CACHE:
        _PROGRAM_CACHE["nc"] = build_program()
    return _PROGRAM_CACHE["nc"]


def _sigma_perm():
    """Pixel permutation induced by the vn gather's 16-wrap + 32x32
    transpose: device pixel-unit (partition q, slot kap) <-> core-local
    image pixel sigma[q*PPP_ + kap]."""
    PPP_ = NP // P
    q = np.arange(P)[:, None]
    kap = np.arange(PPP_)[None, :]
    a = q // 32
    e = (q // 16) % 2
    jp = q % 16
    k2 = kap // 2
    w = kap % 2
    g = 2 * a + w
    k = 2 * k2 + e
    return ((16 * g + jp) * PPP_ + k).reshape(-1)


_SIGMA = _sigma_perm()


def make_in_maps(
    pix_to_face, bary_coords, faces, verts, vertex_normals, directions,
    env_colors, texels,
):
    del verts  # unused by the computation
    in_maps = []
    faces_c = np.ascontiguousarray(faces, dtype=np.int32)
    vnormT_c = np.ascontiguousarray(
        np.asarray(vertex_normals, dtype=np.float32).T
    )
    dirs_c = np.ascontiguousarray(directions, dtype=np.float32).reshape(B * J, 3)
    env_c = np.ascontiguousarray(env_colors, dtype=np.float32).reshape(B * J, 3)
    for r in range(NCORES):
        rs = slice(r * HPC, (r + 1) * HPC)
        bary_r = np.asarray(bary_coords[0, rs, :, 0, :], dtype=np.float32).reshape(
            NP, 3
        )
        tex_r = np.asarray(texels[0, rs, :, :], dtype=np.float32).reshape(NP, 3)
        in_maps.append(
            {
                "p2f": np.ascontiguousarray(
                    pix_to_face[0, rs, :, 0], dtype=np.int32
                ).reshape(NP),
                "bary": np.ascontiguousarray(bary_r[_SIGMA]),
                "texels": np.ascontiguousarray(tex_r[_SIGMA]),
                "faces": faces_c,
                "vnorm_T": vnormT_c,
                "dirs": dirs_c,
                "env": env_c,
            }
        )
    return in_maps


def assemble_outputs(results):
    colors = np.empty((B, H, W, 3), dtype=np.float32)
    diffuse = np.empty((B, H, W, 3), dtype=np.float32)
    normals = np.empty((B, H, W, 3), dtype=np.float32)
    texels = np.empty((1, H, W, 3), dtype=np.float32)
    inv = np.empty_like(_SIGMA)
    inv[_SIGMA] = np.arange(NP)
    for r in range(NCORES):
        rs = slice(r * HPC, (r + 1) * HPC)
        colors[:, rs] = results[r]["colors"][:, inv].reshape(B, HPC, W, 3)
        diffuse[:, rs] = results[r]["diffuse"][:, inv].reshape(B, HPC, W, 3)
        normals[:, rs] = results[r]["normals"][:, inv].reshape(B, HPC, W, 3)
        texels[0, rs] = results[r]["texels_out"][inv].reshape(HPC, W, 3)
    return colors, diffuse, texels, normals


def run_on_hw(in_maps, trace=False):
    nc = _get_program()
    return run_bass_kernel_spmd(nc, in_maps, list(range(NCORES)), trace=trace)


def kernel(**inputs):
    in_maps = make_in_maps(**inputs)
    res = run_on_hw(in_maps)
    return assemble_outputs(res.results)
